# revision 15
# baseline (speedup 1.0000x reference)
"""Sliding-window attention Trainium2 Bass kernel (v3: bf16 + rebalanced).

Problem: B=4, H=32, L=4096, D=128, window=512.
reference: attends over the LAST w=512 key/value positions; query row i may
only see window slot j when j <= i (slots are key positions L-w+j).

Sharding: B*H = 128 (b,h) pairs split across 8 cores -> 16 heads/core.
Pure data parallelism, no collectives.

v3 changes vs v1 (403us):
  - all matmul operands bf16 (host converts); output bf16, host upcasts.
    Same PE stream rate as f32r but halves DMA and enables fast weight load.
  - rowsum: was 4 accumulated [1,512] matmuls (4x512 PE streaming cycles per
    group). Now ONE bf16 DVE op pair-sums the P chunks via strided free-dim
    APs (T2[:,i,:] = P[2i]+P[2i+1], 2x packed mode) and only 2 accumulated
    matmuls stream through the PE. Cuts PE/group from 12 to 10 matmul
    streams and keeps the DVE under the ACT period.
  - exp: one big ACT instr per psum half tile [128,1024] (2/group) -- ACT
    is the pacing engine at ~2.15us/group.
  - group-0 causal masking via an extra upper-triangular matmul accumulate
    (PE) instead of DVE mask adds, plus DVE memsets of the fully-masked
    P rectangles (stale-psum exp garbage -> zeros before rowsum reads).

Per group g (512 queries), steady state emission (3-deep pipeline):
  PE : 4 S^T matmuls (2 per half) + 4 PV + 2 rowsum accumulates
  ACT: 2x exp([128,1024] psum) -> p bf16 sbuf       (~2.15us, bottleneck)
  DVE: T2 pair-sum (1 op), reciprocal_approx of sums row, final
       O^T * (1/sum) psum->sbuf bf16
  GPS: partition_broadcast of the reciprocal row
PSUM: S 2x[128,1024] (4 banks) + O 3x[128,512] (3) + sums [1,512] (1) = 8/8.
"""
import math
from collections import deque
from contextlib import ExitStack

import numpy as np

N_CORES = 8
B, H, L, D = 4, 32, 4096, 128
W = 512            # window
HEADS_PER_CORE = (B * H) // N_CORES   # 16
QG = 512           # queries per group
NG = L // QG       # groups per head (8)
NCHUNK = W // 128  # 4 window chunks
NEG = -1.0e9       # additive mask value (pre-scale)
SCALE = 1.0 / math.sqrt(D)

_COMPILED = None


def _build():
    import concourse.tile as tile
    from concourse import bacc, mybir
    from concourse import bass_isa

    nc = bacc.Bacc("TRN2", target_bir_lowering=False, debug=False,
                   num_devices=N_CORES)

    bf16 = mybir.dt.bfloat16
    f32 = mybir.dt.float32

    qT = nc.dram_tensor("qT", [HEADS_PER_CORE, D, L], bf16, kind="ExternalInput").ap()
    kT = nc.dram_tensor("kT", [HEADS_PER_CORE, D, W], bf16, kind="ExternalInput").ap()
    v = nc.dram_tensor("v", [HEADS_PER_CORE, W, D], bf16, kind="ExternalInput").ap()
    utri = nc.dram_tensor("utri", [128, 128], bf16, kind="ExternalInput").ap()
    ident = nc.dram_tensor("ident", [128, 128], bf16, kind="ExternalInput").ap()
    ones = nc.dram_tensor("ones", [128, 1], bf16, kind="ExternalInput").ap()
    outT = nc.dram_tensor("outT", [HEADS_PER_CORE, D, L], bf16, kind="ExternalOutput").ap()

    with tile.TileContext(nc) as tc:
        with ExitStack() as ctx:
            const = ctx.enter_context(tc.tile_pool(name="const", bufs=1))
            kt_pool = ctx.enter_context(tc.tile_pool(name="kt", bufs=2))
            v_pool = ctx.enter_context(tc.tile_pool(name="v", bufs=2))
            q_pool = ctx.enter_context(tc.tile_pool(name="q", bufs=2 * NG))
            p_pool = ctx.enter_context(tc.tile_pool(name="p", bufs=3))
            t2_pool = ctx.enter_context(tc.tile_pool(name="t2", bufs=2))
            rbc_pool = ctx.enter_context(tc.tile_pool(name="rbc", bufs=2))
            rinv_pool = ctx.enter_context(tc.tile_pool(name="rinv", bufs=2))
            o_pool = ctx.enter_context(tc.tile_pool(name="o", bufs=3))
            s_psum = ctx.enter_context(tc.tile_pool(name="s_ps", bufs=2, space="PSUM"))
            o_psum = ctx.enter_context(tc.tile_pool(name="o_ps", bufs=3, space="PSUM"))
            sum_psum = ctx.enter_context(tc.tile_pool(name="sm_ps", bufs=1, space="PSUM"))

            utri_t = const.tile([128, 128], bf16, tag="utri")
            nc.gpsimd.dma_start(utri_t[:], utri[:])
            ident_t = const.tile([128, 128], bf16, tag="ident")
            nc.gpsimd.dma_start(ident_t[:], ident[:])
            ones_t = const.tile([128, 1], bf16, tag="ones")
            nc.gpsimd.dma_start(ones_t[:], ones[:])


            head_tiles = {}

            def load_head_main(h):
                # kt + q0 first: they gate the head's first S matmuls.
                # kt goes via the gpsimd DGE queue so it runs in parallel
                # with q0 on the sync queue.
                kt_t = kt_pool.tile([128, W], bf16, tag="kt")
                nc.gpsimd.dma_start(kt_t[:], kT[h])
                qt_t0 = q_pool.tile([128, QG], bf16, tag="q")
                nc.sync.dma_start(qt_t0[:], qT[h, :, 0:QG])
                v_t = v_pool.tile([128, NCHUNK * D], bf16, tag="v")
                for c in range(NCHUNK):
                    nc.sync.dma_start(v_t[:, c * D:(c + 1) * D],
                                      v[h, c * 128:(c + 1) * 128, :])
                head_tiles[h] = (kt_t, v_t, [qt_t0])

            def load_q(h, i):
                qt_t = q_pool.tile([128, QG], bf16, tag="q")
                nc.sync.dma_start(qt_t[:], qT[h, :, i * QG:(i + 1) * QG])
                head_tiles[h][2].append(qt_t)

            def emit_s_half(h, g, p_t, half):
                """S matmuls + mask + exp for one half of group (h, g)."""
                kt_t, v_t, qt_tiles = head_tiles[h]
                qt_t = qt_tiles[g]
                s_ps = s_psum.tile([128, 2 * QG], f32, tag="s")
                for ci in range(2):
                    c = half * 2 + ci
                    # Group 0: queries < c*128 can't see chunk c. Shrink the
                    # matmul; the stale psum there gets exp'd to (finite)
                    # garbage and memset to 0 in P below. Not at h==0 where
                    # the stale psum could be inf/nan garbage.
                    q_lo = c * 128 if (g == 0 and h > 0) else 0
                    nc.tensor.matmul(
                        s_ps[:, ci * QG + q_lo:(ci + 1) * QG],
                        lhsT=kt_t[:, c * 128:(c + 1) * 128],
                        rhs=qt_t[:, q_lo:QG],
                        start=True, stop=True,
                    )
                if g == 0:
                    # add the strictly-upper-triangular -1e9 mask onto the
                    # diagonal 128x128 block of each chunk via the PE:
                    # out[w,q] += utri[q,w] (utri = lhsT, rhs = identity).
                    for ci in range(2):
                        c = half * 2 + ci
                        blk = slice(ci * QG + c * 128, ci * QG + (c + 1) * 128)
                        nc.tensor.matmul(
                            s_ps[:, blk],
                            lhsT=utri_t[:],
                            rhs=ident_t[:],
                            start=False, stop=True,
                            skip_group_check=True,
                        )
                nc.scalar.activation(
                    p_t[:, 2 * half:2 * half + 2, :],
                    s_ps[:],
                    mybir.ActivationFunctionType.Exp, scale=SCALE)

            def emit_t2(stage):
                """pair-sum P chunks on DVE (one bf16 op, strided APs)."""
                h, g, p_t = stage
                t2 = t2_pool.tile([128, 2, QG], bf16, tag="t2")
                nc.vector.tensor_add(t2[:], p_t[:, 0:NCHUNK:2, :],
                                     p_t[:, 1:NCHUNK:2, :])
                return t2

            def emit_pv(stage, c0, c1):
                """PV matmul chunks [c0, c1) accumulated into o_ps."""
                h, g, p_t, o_ps, t2 = stage
                kt_t, v_t, qt_tiles = head_tiles[h]
                for c in range(c0, c1):
                    q_lo = c * 128 if g == 0 else 0
                    nc.tensor.matmul(
                        o_ps[:, q_lo:QG],
                        lhsT=v_t[:, c * D:(c + 1) * D],
                        rhs=p_t[:, c, q_lo:QG],
                        start=(c == 0), stop=(c == NCHUNK - 1),
                    )

            def emit_rs(stage):
                """2-matmul rowsum of the pair-sums."""
                h, g, p_t, o_ps, t2 = stage
                sums_ps = sum_psum.tile([1, QG], f32, tag="sums")
                for i in range(2):
                    nc.tensor.matmul(
                        sums_ps[:],
                        lhsT=ones_t[:],
                        rhs=t2[:, i, :],
                        start=(i == 0), stop=(i == 1),
                    )
                return (h, g, o_ps, sums_ps)

            def emit_back_gps(stage2):
                """reciprocal of the sums row + broadcast to 128 partitions."""
                h, g, o_ps, sums_ps = stage2
                rinv_t = rinv_pool.tile([1, QG], f32, tag="rinv")
                nc.vector.reciprocal_approx_fast(rinv_t[:], sums_ps[:])
                rbc_t = rbc_pool.tile([128, QG], f32, tag="rbc")
                nc.gpsimd.partition_broadcast(rbc_t[:], rinv_t[:])
                return (h, g, o_ps, rbc_t)

            def emit_back_norm(stage3):
                """normalize + store."""
                h, g, o_ps, rbc_t = stage3
                o_t = o_pool.tile([128, QG], bf16, tag="o")
                nc.vector.tensor_mul(o_t[:], o_ps[:], rbc_t[:])
                # store via the (mostly idle) GPSIMD DGE queue so head-load
                # bursts on the sync queue never starve the output path
                nc.gpsimd.dma_start(outT[h, :, g * QG:(g + 1) * QG], o_t[:])

            TOTAL = HEADS_PER_CORE * NG
            prev = None       # back stage being interleaved this iteration
            gps_q = deque()   # rowsum stages awaiting recip/broadcast
            norm_q = deque()  # broadcast stages awaiting final mul

            load_head_main(0)
            for i in range(1, NG):
                load_q(0, i)
            for it in range(TOTAL):
                h, g = divmod(it, NG)
                if prev is not None:
                    # pair-sum FIRST in the DVE queue: the rowsum matmuls
                    # mid-iteration depend on it
                    ph, pg, pp = prev
                    po = o_psum.tile([128, QG], f32, tag="ops")
                    pt2 = emit_t2(prev)
                    prev_full = (ph, pg, pp, po, pt2)
                if len(norm_q) > 1:
                    # final mul early so the O-psum buffer frees before this
                    # iteration's PV matmuls need it
                    emit_back_norm(norm_q.popleft())
                if h + 1 < HEADS_PER_CORE:
                    # prefetch the next head spread over iterations g=4..7
                    # (one burst of 13 DMAs would starve the output queue)
                    if g == NG // 2:
                        load_head_main(h + 1)
                        load_q(h + 1, 1)
                    elif g > NG // 2:
                        load_q(h + 1, 2 * (g - NG // 2))
                        load_q(h + 1, 2 * (g - NG // 2) + 1)
                # PE schedule: [S.h0 + exp.h0] [PV c0,c1] [S.h1 + exp.h1]
                # [rs x2] [PV c2,c3] -- each exp's semaphore lands right
                # after its own half's matmuls, the PV/rs stream keeps the
                # PE busy while ACT runs, and nothing slow sits between the
                # PV tail and the next iteration's S matmuls.
                p_t = p_pool.tile([128, NCHUNK, QG], bf16, tag="p")
                emit_s_half(h, g, p_t, 0)
                if prev is not None:
                    emit_pv(prev_full, 0, 2)
                emit_s_half(h, g, p_t, 1)
                if g == 0:
                    # zero the fully-masked rectangles (queries < c*128 of
                    # chunk c) so the full-width rowsum reads zeros there.
                    for c in range(1, NCHUNK):
                        nc.vector.memset(p_t[:, c, 0:c * 128], 0.0)
                if prev is not None:
                    gps_q.append(emit_rs(prev_full))
                    emit_pv(prev_full, 2, NCHUNK)
                    if pg == NG - 1:
                        del head_tiles[ph]
                if len(gps_q) > 1:
                    norm_q.append(emit_back_gps(gps_q.popleft()))
                prev = (h, g, p_t)
            ph, pg, pp = prev
            po = o_psum.tile([128, QG], f32, tag="ops")
            pt2 = emit_t2(prev)
            prev_full = (ph, pg, pp, po, pt2)
            emit_pv(prev_full, 0, NCHUNK)
            gps_q.append(emit_rs(prev_full))
            while gps_q:
                norm_q.append(emit_back_gps(gps_q.popleft()))
            while norm_q:
                emit_back_norm(norm_q.popleft())

    nc.compile()
    return nc


def _get_compiled():
    global _COMPILED
    if _COMPILED is None:
        _COMPILED = _build()
    return _COMPILED


def _make_in_maps(query, keys, values):
    from ml_dtypes import bfloat16

    q = np.asarray(query, dtype=np.float32)
    k = np.asarray(keys, dtype=np.float32)
    v = np.asarray(values, dtype=np.float32)

    qf = q.reshape(B * H, L, D)
    kf = k.reshape(B * H, L, D)[:, L - W:, :]
    vf = v.reshape(B * H, L, D)[:, L - W:, :]

    # strictly-upper-triangular additive mask block, [q, w] indexed:
    # utri[q, w] = NEG where q < w (as matmul lhsT it lands as out[w,q] += NEG)
    utri = np.where(np.arange(128)[:, None] < np.arange(128)[None, :],
                    np.float32(NEG), np.float32(0.0)).astype(bfloat16)
    ident = np.eye(128, dtype=np.float32).astype(bfloat16)
    ones = np.ones((128, 1), dtype=np.float32).astype(bfloat16)

    in_maps = []
    for core in range(N_CORES):
        s = slice(core * HEADS_PER_CORE, (core + 1) * HEADS_PER_CORE)
        in_maps.append({
            "qT": np.ascontiguousarray(qf[s].transpose(0, 2, 1)).astype(bfloat16),
            "kT": np.ascontiguousarray(kf[s].transpose(0, 2, 1)).astype(bfloat16),
            "v": np.ascontiguousarray(vf[s]).astype(bfloat16),
            "utri": utri,
            "ident": ident,
            "ones": ones,
        })
    return in_maps


def kernel(query, keys, values, window_size):
    from concourse.bass_utils import run_bass_kernel_spmd

    w = int(window_size)
    assert np.asarray(query).shape == (B, H, L, D) and w == W

    nc = _get_compiled()
    in_maps = _make_in_maps(query, keys, values)
    res = run_bass_kernel_spmd(nc, in_maps, core_ids=list(range(N_CORES)))
    outs = [np.asarray(res.results[c]["outT"], dtype=np.float32).transpose(0, 2, 1)
            for c in range(N_CORES)]
    return np.concatenate(outs, axis=0).reshape(B, H, L, D)


# revision 16
# speedup vs baseline: 1.0295x; 1.0295x over previous
"""Sliding-window attention Trainium2 Bass kernel (v3: bf16 + rebalanced).

Problem: B=4, H=32, L=4096, D=128, window=512.
reference: attends over the LAST w=512 key/value positions; query row i may
only see window slot j when j <= i (slots are key positions L-w+j).

Sharding: B*H = 128 (b,h) pairs split across 8 cores -> 16 heads/core.
Pure data parallelism, no collectives.

v3 changes vs v1 (403us):
  - all matmul operands bf16 (host converts); output bf16, host upcasts.
    Same PE stream rate as f32r but halves DMA and enables fast weight load.
  - rowsum: was 4 accumulated [1,512] matmuls (4x512 PE streaming cycles per
    group). Now ONE bf16 DVE op pair-sums the P chunks via strided free-dim
    APs (T2[:,i,:] = P[2i]+P[2i+1], 2x packed mode) and only 2 accumulated
    matmuls stream through the PE. Cuts PE/group from 12 to 10 matmul
    streams and keeps the DVE under the ACT period.
  - exp: one big ACT instr per psum half tile [128,1024] (2/group) -- ACT
    is the pacing engine at ~2.15us/group.
  - group-0 causal masking via an extra upper-triangular matmul accumulate
    (PE) instead of DVE mask adds, plus DVE memsets of the fully-masked
    P rectangles (stale-psum exp garbage -> zeros before rowsum reads).

Per group g (512 queries), steady state emission (3-deep pipeline):
  PE : 4 S^T matmuls (2 per half) + 4 PV + 2 rowsum accumulates
  ACT: 2x exp([128,1024] psum) -> p bf16 sbuf       (~2.15us, bottleneck)
  DVE: T2 pair-sum (1 op), reciprocal_approx of sums row, final
       O^T * (1/sum) psum->sbuf bf16
  GPS: partition_broadcast of the reciprocal row
PSUM: S 2x[128,1024] (4 banks) + O 3x[128,512] (3) + sums [1,512] (1) = 8/8.
"""
import math
from collections import deque
from contextlib import ExitStack

import numpy as np

N_CORES = 8
B, H, L, D = 4, 32, 4096, 128
W = 512            # window
HEADS_PER_CORE = (B * H) // N_CORES   # 16
QG = 512           # queries per group
NG = L // QG       # groups per head (8)
NCHUNK = W // 128  # 4 window chunks
NEG = -1.0e9       # additive mask value (pre-scale)
SCALE = 1.0 / math.sqrt(D)

_COMPILED = None


def _build():
    import concourse.tile as tile
    from concourse import bacc, mybir
    from concourse import bass_isa

    nc = bacc.Bacc("TRN2", target_bir_lowering=False, debug=False,
                   num_devices=N_CORES)

    bf16 = mybir.dt.bfloat16
    f32 = mybir.dt.float32

    qT = nc.dram_tensor("qT", [HEADS_PER_CORE, D, L], bf16, kind="ExternalInput").ap()
    kT = nc.dram_tensor("kT", [HEADS_PER_CORE, D, W], bf16, kind="ExternalInput").ap()
    v = nc.dram_tensor("v", [HEADS_PER_CORE, W, D], bf16, kind="ExternalInput").ap()
    utri = nc.dram_tensor("utri", [128, 128], bf16, kind="ExternalInput").ap()
    ident = nc.dram_tensor("ident", [128, 128], bf16, kind="ExternalInput").ap()
    ones = nc.dram_tensor("ones", [128, 1], bf16, kind="ExternalInput").ap()
    outT = nc.dram_tensor("outT", [HEADS_PER_CORE, D, L], bf16, kind="ExternalOutput").ap()

    with tile.TileContext(nc) as tc:
        with ExitStack() as ctx:
            const = ctx.enter_context(tc.tile_pool(name="const", bufs=1))
            kt_pool = ctx.enter_context(tc.tile_pool(name="kt", bufs=2))
            v_pool = ctx.enter_context(tc.tile_pool(name="v", bufs=2))
            q_pool = ctx.enter_context(tc.tile_pool(name="q", bufs=2 * NG))
            p_pool = ctx.enter_context(tc.tile_pool(name="p", bufs=3))
            t2_pool = ctx.enter_context(tc.tile_pool(name="t2", bufs=2))
            rbc_pool = ctx.enter_context(tc.tile_pool(name="rbc", bufs=2))
            rinv_pool = ctx.enter_context(tc.tile_pool(name="rinv", bufs=2))
            o_pool = ctx.enter_context(tc.tile_pool(name="o", bufs=3))
            s_psum = ctx.enter_context(tc.tile_pool(name="s_ps", bufs=2, space="PSUM"))
            o_psum = ctx.enter_context(tc.tile_pool(name="o_ps", bufs=3, space="PSUM"))
            sum_psum = ctx.enter_context(tc.tile_pool(name="sm_ps", bufs=1, space="PSUM"))

            utri_t = const.tile([128, 128], bf16, tag="utri")
            nc.gpsimd.dma_start(utri_t[:], utri[:])
            ident_t = const.tile([128, 128], bf16, tag="ident")
            nc.gpsimd.dma_start(ident_t[:], ident[:])
            ones_t = const.tile([128, 1], bf16, tag="ones")
            nc.gpsimd.dma_start(ones_t[:], ones[:])


            head_tiles = {}

            def load_head_main(h):
                # kt + q0 first: they gate the head's first S matmuls
                kt_t = kt_pool.tile([128, W], bf16, tag="kt")
                nc.sync.dma_start(kt_t[:], kT[h])
                qt_t0 = q_pool.tile([128, QG], bf16, tag="q")
                nc.sync.dma_start(qt_t0[:], qT[h, :, 0:QG])
                v_t = v_pool.tile([128, NCHUNK * D], bf16, tag="v")
                for c in range(NCHUNK):
                    nc.sync.dma_start(v_t[:, c * D:(c + 1) * D],
                                      v[h, c * 128:(c + 1) * 128, :])
                head_tiles[h] = (kt_t, v_t, [qt_t0])

            def load_q(h, i):
                qt_t = q_pool.tile([128, QG], bf16, tag="q")
                nc.sync.dma_start(qt_t[:], qT[h, :, i * QG:(i + 1) * QG])
                head_tiles[h][2].append(qt_t)

            def emit_s_half(h, g, p_t, half):
                """S matmuls + mask + exp for one half of group (h, g)."""
                kt_t, v_t, qt_tiles = head_tiles[h]
                qt_t = qt_tiles[g]
                s_ps = s_psum.tile([128, 2 * QG], f32, tag="s")
                for ci in range(2):
                    c = half * 2 + ci
                    # Group 0: queries < c*128 can't see chunk c. Shrink the
                    # matmul; the stale psum there gets exp'd to (finite)
                    # garbage and memset to 0 in P below. Not at h==0 where
                    # the stale psum could be inf/nan garbage.
                    q_lo = c * 128 if (g == 0 and h > 0) else 0
                    nc.tensor.matmul(
                        s_ps[:, ci * QG + q_lo:(ci + 1) * QG],
                        lhsT=kt_t[:, c * 128:(c + 1) * 128],
                        rhs=qt_t[:, q_lo:QG],
                        start=True, stop=True,
                    )
                if g == 0:
                    # add the strictly-upper-triangular -1e9 mask onto the
                    # diagonal 128x128 block of each chunk via the PE:
                    # out[w,q] += utri[q,w] (utri = lhsT, rhs = identity).
                    for ci in range(2):
                        c = half * 2 + ci
                        blk = slice(ci * QG + c * 128, ci * QG + (c + 1) * 128)
                        nc.tensor.matmul(
                            s_ps[:, blk],
                            lhsT=utri_t[:],
                            rhs=ident_t[:],
                            start=False, stop=True,
                            skip_group_check=True,
                        )
                nc.scalar.activation(
                    p_t[:, 2 * half:2 * half + 2, :],
                    s_ps[:],
                    mybir.ActivationFunctionType.Exp, scale=SCALE)

            def emit_t2(stage):
                """pair-sum P chunks on DVE (one bf16 op, strided APs)."""
                h, g, p_t = stage
                t2 = t2_pool.tile([128, 2, QG], bf16, tag="t2")
                nc.vector.tensor_add(t2[:], p_t[:, 0:NCHUNK:2, :],
                                     p_t[:, 1:NCHUNK:2, :])
                return t2

            def emit_pv(stage, c0, c1):
                """PV matmul chunks [c0, c1) accumulated into o_ps."""
                h, g, p_t, o_ps, t2 = stage
                kt_t, v_t, qt_tiles = head_tiles[h]
                for c in range(c0, c1):
                    q_lo = c * 128 if g == 0 else 0
                    nc.tensor.matmul(
                        o_ps[:, q_lo:QG],
                        lhsT=v_t[:, c * D:(c + 1) * D],
                        rhs=p_t[:, c, q_lo:QG],
                        start=(c == 0), stop=(c == NCHUNK - 1),
                    )

            def emit_rs(stage):
                """2-matmul rowsum of the pair-sums."""
                h, g, p_t, o_ps, t2 = stage
                sums_ps = sum_psum.tile([1, QG], f32, tag="sums")
                for i in range(2):
                    nc.tensor.matmul(
                        sums_ps[:],
                        lhsT=ones_t[:],
                        rhs=t2[:, i, :],
                        start=(i == 0), stop=(i == 1),
                    )
                return (h, g, o_ps, sums_ps)

            def emit_back_gps(stage2):
                """reciprocal of the sums row + broadcast to 128 partitions."""
                h, g, o_ps, sums_ps = stage2
                rinv_t = rinv_pool.tile([1, QG], f32, tag="rinv")
                nc.vector.reciprocal_approx_fast(rinv_t[:], sums_ps[:])
                rbc_t = rbc_pool.tile([128, QG], f32, tag="rbc")
                nc.gpsimd.partition_broadcast(rbc_t[:], rinv_t[:])
                return (h, g, o_ps, rbc_t)

            def emit_back_norm(stage3):
                """normalize + store."""
                h, g, o_ps, rbc_t = stage3
                o_t = o_pool.tile([128, QG], bf16, tag="o")
                nc.vector.tensor_mul(o_t[:], o_ps[:], rbc_t[:])
                nc.sync.dma_start(outT[h, :, g * QG:(g + 1) * QG], o_t[:])

            TOTAL = HEADS_PER_CORE * NG
            prev = None       # back stage being interleaved this iteration
            gps_q = deque()   # rowsum stages awaiting recip/broadcast
            norm_q = deque()  # broadcast stages awaiting final mul

            load_head_main(0)
            for i in range(1, NG):
                load_q(0, i)
            for it in range(TOTAL):
                h, g = divmod(it, NG)
                if prev is not None:
                    # pair-sum FIRST in the DVE queue: the rowsum matmuls
                    # mid-iteration depend on it
                    ph, pg, pp = prev
                    po = o_psum.tile([128, QG], f32, tag="ops")
                    pt2 = emit_t2(prev)
                    prev_full = (ph, pg, pp, po, pt2)
                if len(norm_q) > 1:
                    # final mul early so the O-psum buffer frees before this
                    # iteration's PV matmuls need it
                    emit_back_norm(norm_q.popleft())
                if h + 1 < HEADS_PER_CORE:
                    # prefetch the next head spread over iterations g=4..7
                    # (one burst of 13 DMAs would starve the output queue)
                    if g == NG // 2:
                        load_head_main(h + 1)
                        load_q(h + 1, 1)
                    elif g > NG // 2:
                        load_q(h + 1, 2 * (g - NG // 2))
                        load_q(h + 1, 2 * (g - NG // 2) + 1)
                # PE schedule: [S.h0 + exp.h0] [PV c0,c1] [S.h1 + exp.h1]
                # [rs x2] [PV c2,c3] -- each exp's semaphore lands right
                # after its own half's matmuls, the PV/rs stream keeps the
                # PE busy while ACT runs, and nothing slow sits between the
                # PV tail and the next iteration's S matmuls.
                p_t = p_pool.tile([128, NCHUNK, QG], bf16, tag="p")
                emit_s_half(h, g, p_t, 0)
                if prev is not None:
                    emit_pv(prev_full, 0, 2)
                emit_s_half(h, g, p_t, 1)
                if g == 0:
                    # zero the fully-masked rectangles (queries < c*128 of
                    # chunk c) so the full-width rowsum reads zeros there.
                    for c in range(1, NCHUNK):
                        nc.vector.memset(p_t[:, c, 0:c * 128], 0.0)
                if prev is not None:
                    gps_q.append(emit_rs(prev_full))
                    emit_pv(prev_full, 2, NCHUNK)
                    if pg == NG - 1:
                        del head_tiles[ph]
                if len(gps_q) > 1:
                    norm_q.append(emit_back_gps(gps_q.popleft()))
                prev = (h, g, p_t)
            ph, pg, pp = prev
            po = o_psum.tile([128, QG], f32, tag="ops")
            pt2 = emit_t2(prev)
            prev_full = (ph, pg, pp, po, pt2)
            emit_pv(prev_full, 0, NCHUNK)
            gps_q.append(emit_rs(prev_full))
            while gps_q:
                norm_q.append(emit_back_gps(gps_q.popleft()))
            while norm_q:
                emit_back_norm(norm_q.popleft())

    nc.compile()
    return nc


def _get_compiled():
    global _COMPILED
    if _COMPILED is None:
        _COMPILED = _build()
    return _COMPILED


def _make_in_maps(query, keys, values):
    from ml_dtypes import bfloat16

    q = np.asarray(query, dtype=np.float32)
    k = np.asarray(keys, dtype=np.float32)
    v = np.asarray(values, dtype=np.float32)

    qf = q.reshape(B * H, L, D)
    kf = k.reshape(B * H, L, D)[:, L - W:, :]
    vf = v.reshape(B * H, L, D)[:, L - W:, :]

    # strictly-upper-triangular additive mask block, [q, w] indexed:
    # utri[q, w] = NEG where q < w (as matmul lhsT it lands as out[w,q] += NEG)
    utri = np.where(np.arange(128)[:, None] < np.arange(128)[None, :],
                    np.float32(NEG), np.float32(0.0)).astype(bfloat16)
    ident = np.eye(128, dtype=np.float32).astype(bfloat16)
    ones = np.ones((128, 1), dtype=np.float32).astype(bfloat16)

    in_maps = []
    for core in range(N_CORES):
        s = slice(core * HEADS_PER_CORE, (core + 1) * HEADS_PER_CORE)
        in_maps.append({
            "qT": np.ascontiguousarray(qf[s].transpose(0, 2, 1)).astype(bfloat16),
            "kT": np.ascontiguousarray(kf[s].transpose(0, 2, 1)).astype(bfloat16),
            "v": np.ascontiguousarray(vf[s]).astype(bfloat16),
            "utri": utri,
            "ident": ident,
            "ones": ones,
        })
    return in_maps


def kernel(query, keys, values, window_size):
    from concourse.bass_utils import run_bass_kernel_spmd

    w = int(window_size)
    assert np.asarray(query).shape == (B, H, L, D) and w == W

    nc = _get_compiled()
    in_maps = _make_in_maps(query, keys, values)
    res = run_bass_kernel_spmd(nc, in_maps, core_ids=list(range(N_CORES)))
    outs = [np.asarray(res.results[c]["outT"], dtype=np.float32).transpose(0, 2, 1)
            for c in range(N_CORES)]
    return np.concatenate(outs, axis=0).reshape(B, H, L, D)


# revision 19
# speedup vs baseline: 1.0341x; 1.0045x over previous
"""Sliding-window attention Trainium2 Bass kernel (bf16, rebalanced, 350us).

Problem: B=4, H=32, L=4096, D=128, window=512.
reference: attends over the LAST w=512 key/value positions; query row i may
only see window slot j when j <= i (slots are key positions L-w+j).

Sharding: B*H = 128 (b,h) pairs split across 8 cores -> 16 heads/core.
Pure data parallelism, no collectives.

Changes vs the 403us f32r baseline:
  - all matmul operands bf16 (host converts); output bf16, host upcasts.
    Same PE stream rate as f32r but halves DMA and enables fast weight load.
  - rowsum: was 4 accumulated [1,512] matmuls (4x512 PE streaming cycles per
    group). Now ONE bf16 DVE op pair-sums the P chunks via strided free-dim
    APs (T2[:,i,:] = P[2i]+P[2i+1], 2x packed mode) and only 2 accumulated
    matmuls stream through the PE. Cuts PE/group from 12 to 10 matmul
    streams and keeps the DVE under the ACT period.
  - exp: one big ACT instr per psum half tile [128,1024] (2/group) -- ACT
    is the pacing engine at ~2.15us/group.
  - group-0 causal masking via an extra upper-triangular matmul accumulate
    (PE) instead of DVE mask adds, plus DVE memsets of the fully-masked
    P rectangles (stale-psum exp garbage -> zeros before rowsum reads).
  - PV matmuls interleaved BETWEEN the two S halves so each exp's input
    semaphore fires right after its own 2 S matmuls; rowsum matmuls sit
    between S.h1 and the PV tail so they never gate the next group's S.
  - next-head prefetch spread over iterations g=4..7 (2 q-tiles each);
    a single 13-DMA burst on the sync queue starved the output DMAs
    (+8us measured). Output DMAs stay on the sync queue: routing them via
    the gpsimd DGE queue delayed the critical partition_broadcast (-7us).

Per group g (512 queries), steady state emission (3-deep pipeline):
  PE : 4 S^T matmuls (2 per half) + 4 PV + 2 rowsum accumulates
  ACT: 2x exp([128,1024] psum) -> p bf16 sbuf       (~2.15us, bottleneck)
  DVE: T2 pair-sum (1 op), reciprocal_approx of sums row, final
       O^T * (1/sum) psum->sbuf bf16
  GPS: partition_broadcast of the reciprocal row
PSUM: S 2x[128,1024] (4 banks) + O 3x[128,512] (3) + sums [1,512] (1) = 8/8.
"""
import math
from collections import deque
from contextlib import ExitStack

import numpy as np

N_CORES = 8
B, H, L, D = 4, 32, 4096, 128
W = 512            # window
HEADS_PER_CORE = (B * H) // N_CORES   # 16
QG = 512           # queries per group
NG = L // QG       # groups per head (8)
NCHUNK = W // 128  # 4 window chunks
NEG = -1.0e9       # additive mask value (pre-scale)
SCALE = 1.0 / math.sqrt(D)

_COMPILED = None


def _build():
    import concourse.tile as tile
    from concourse import bacc, mybir
    from concourse import bass_isa

    nc = bacc.Bacc("TRN2", target_bir_lowering=False, debug=False,
                   num_devices=N_CORES)

    bf16 = mybir.dt.bfloat16
    f32 = mybir.dt.float32

    qT = nc.dram_tensor("qT", [HEADS_PER_CORE, D, L], bf16, kind="ExternalInput").ap()
    kT = nc.dram_tensor("kT", [HEADS_PER_CORE, D, W], bf16, kind="ExternalInput").ap()
    v = nc.dram_tensor("v", [HEADS_PER_CORE, W, D], bf16, kind="ExternalInput").ap()
    utri = nc.dram_tensor("utri", [128, 128], bf16, kind="ExternalInput").ap()
    ident = nc.dram_tensor("ident", [128, 128], bf16, kind="ExternalInput").ap()
    ones = nc.dram_tensor("ones", [128, 1], bf16, kind="ExternalInput").ap()
    outT = nc.dram_tensor("outT", [HEADS_PER_CORE, D, L], bf16, kind="ExternalOutput").ap()

    with tile.TileContext(nc) as tc:
        with ExitStack() as ctx:
            const = ctx.enter_context(tc.tile_pool(name="const", bufs=1))
            kt_pool = ctx.enter_context(tc.tile_pool(name="kt", bufs=2))
            v_pool = ctx.enter_context(tc.tile_pool(name="v", bufs=2))
            q_pool = ctx.enter_context(tc.tile_pool(name="q", bufs=2 * NG))
            p_pool = ctx.enter_context(tc.tile_pool(name="p", bufs=4))
            t2_pool = ctx.enter_context(tc.tile_pool(name="t2", bufs=3))
            rbc_pool = ctx.enter_context(tc.tile_pool(name="rbc", bufs=3))
            rinv_pool = ctx.enter_context(tc.tile_pool(name="rinv", bufs=3))
            o_pool = ctx.enter_context(tc.tile_pool(name="o", bufs=3))
            s_psum = ctx.enter_context(tc.tile_pool(name="s_ps", bufs=2, space="PSUM"))
            o_psum = ctx.enter_context(tc.tile_pool(name="o_ps", bufs=3, space="PSUM"))
            sum_psum = ctx.enter_context(tc.tile_pool(name="sm_ps", bufs=1, space="PSUM"))

            utri_t = const.tile([128, 128], bf16, tag="utri")
            nc.gpsimd.dma_start(utri_t[:], utri[:])
            ident_t = const.tile([128, 128], bf16, tag="ident")
            nc.gpsimd.dma_start(ident_t[:], ident[:])
            ones_t = const.tile([128, 1], bf16, tag="ones")
            nc.gpsimd.dma_start(ones_t[:], ones[:])


            head_tiles = {}

            def load_head_main(h):
                # kt + q0 first: they gate the head's first S matmuls
                kt_t = kt_pool.tile([128, W], bf16, tag="kt")
                nc.sync.dma_start(kt_t[:], kT[h])
                qt_t0 = q_pool.tile([128, QG], bf16, tag="q")
                nc.sync.dma_start(qt_t0[:], qT[h, :, 0:QG])
                v_t = v_pool.tile([128, NCHUNK * D], bf16, tag="v")
                for c in range(NCHUNK):
                    nc.sync.dma_start(v_t[:, c * D:(c + 1) * D],
                                      v[h, c * 128:(c + 1) * 128, :])
                head_tiles[h] = (kt_t, v_t, [qt_t0])

            def load_q(h, i):
                qt_t = q_pool.tile([128, QG], bf16, tag="q")
                nc.sync.dma_start(qt_t[:], qT[h, :, i * QG:(i + 1) * QG])
                head_tiles[h][2].append(qt_t)

            def emit_s_half(h, g, p_t, half):
                """S matmuls + mask + exp for one half of group (h, g)."""
                kt_t, v_t, qt_tiles = head_tiles[h]
                qt_t = qt_tiles[g]
                s_ps = s_psum.tile([128, 2 * QG], f32, tag="s")
                for ci in range(2):
                    c = half * 2 + ci
                    # Group 0: queries < c*128 can't see chunk c. Shrink the
                    # matmul; the stale psum there gets exp'd to (finite)
                    # garbage and memset to 0 in P below. Not at h==0 where
                    # the stale psum could be inf/nan garbage.
                    q_lo = c * 128 if (g == 0 and h > 0) else 0
                    nc.tensor.matmul(
                        s_ps[:, ci * QG + q_lo:(ci + 1) * QG],
                        lhsT=kt_t[:, c * 128:(c + 1) * 128],
                        rhs=qt_t[:, q_lo:QG],
                        start=True, stop=True,
                    )
                if g == 0:
                    # add the strictly-upper-triangular -1e9 mask onto the
                    # diagonal 128x128 block of each chunk via the PE:
                    # out[w,q] += utri[q,w] (utri = lhsT, rhs = identity).
                    for ci in range(2):
                        c = half * 2 + ci
                        blk = slice(ci * QG + c * 128, ci * QG + (c + 1) * 128)
                        nc.tensor.matmul(
                            s_ps[:, blk],
                            lhsT=utri_t[:],
                            rhs=ident_t[:],
                            start=False, stop=True,
                            skip_group_check=True,
                        )
                nc.scalar.activation(
                    p_t[:, 2 * half:2 * half + 2, :],
                    s_ps[:],
                    mybir.ActivationFunctionType.Exp, scale=SCALE)

            def emit_t2(stage):
                """pair-sum P chunks on DVE (one bf16 op, strided APs)."""
                h, g, p_t = stage
                t2 = t2_pool.tile([128, 2, QG], bf16, tag="t2")
                nc.vector.tensor_add(t2[:], p_t[:, 0:NCHUNK:2, :],
                                     p_t[:, 1:NCHUNK:2, :])
                return t2

            def emit_pv(stage, c0, c1):
                """PV matmul chunks [c0, c1) accumulated into o_ps."""
                h, g, p_t, o_ps, t2 = stage
                kt_t, v_t, qt_tiles = head_tiles[h]
                for c in range(c0, c1):
                    q_lo = c * 128 if g == 0 else 0
                    nc.tensor.matmul(
                        o_ps[:, q_lo:QG],
                        lhsT=v_t[:, c * D:(c + 1) * D],
                        rhs=p_t[:, c, q_lo:QG],
                        start=(c == 0), stop=(c == NCHUNK - 1),
                    )

            def emit_rs(stage):
                """2-matmul rowsum of the pair-sums."""
                h, g, p_t, o_ps, t2 = stage
                sums_ps = sum_psum.tile([1, QG], f32, tag="sums")
                for i in range(2):
                    nc.tensor.matmul(
                        sums_ps[:],
                        lhsT=ones_t[:],
                        rhs=t2[:, i, :],
                        start=(i == 0), stop=(i == 1),
                    )
                return (h, g, o_ps, sums_ps)

            def emit_back_gps(stage2):
                """reciprocal of the sums row + broadcast to 128 partitions."""
                h, g, o_ps, sums_ps = stage2
                rinv_t = rinv_pool.tile([1, QG], f32, tag="rinv")
                nc.vector.reciprocal_approx_fast(rinv_t[:], sums_ps[:])
                rbc_t = rbc_pool.tile([128, QG], f32, tag="rbc")
                nc.gpsimd.partition_broadcast(rbc_t[:], rinv_t[:])
                return (h, g, o_ps, rbc_t)

            def emit_back_norm(stage3):
                """normalize + store."""
                h, g, o_ps, rbc_t = stage3
                o_t = o_pool.tile([128, QG], bf16, tag="o")
                nc.vector.tensor_mul(o_t[:], o_ps[:], rbc_t[:])
                nc.sync.dma_start(outT[h, :, g * QG:(g + 1) * QG], o_t[:])

            TOTAL = HEADS_PER_CORE * NG
            prev = None       # back stage being interleaved this iteration
            gps_q = deque()   # rowsum stages awaiting recip/broadcast
            norm_q = deque()  # broadcast stages awaiting final mul

            load_head_main(0)
            for i in range(1, NG):
                load_q(0, i)
            for it in range(TOTAL):
                h, g = divmod(it, NG)
                if prev is not None:
                    # pair-sum FIRST in the DVE queue: the rowsum matmuls
                    # mid-iteration depend on it
                    ph, pg, pp = prev
                    po = o_psum.tile([128, QG], f32, tag="ops")
                    pt2 = emit_t2(prev)
                    prev_full = (ph, pg, pp, po, pt2)
                if gps_q:
                    # recip (it-2) ahead of mul in the DVE queue: it is the
                    # WAR release for the single-bank sums psum that this
                    # iteration's rowsum matmuls overwrite
                    norm_q.append(emit_back_gps(gps_q.popleft()))
                if len(norm_q) > 1:
                    emit_back_norm(norm_q.popleft())
                if h + 1 < HEADS_PER_CORE:
                    # prefetch the next head spread over iterations g=4..7
                    # (one burst of 13 DMAs would starve the output queue)
                    if g == NG // 2:
                        load_head_main(h + 1)
                        load_q(h + 1, 1)
                    elif g > NG // 2:
                        load_q(h + 1, 2 * (g - NG // 2))
                        load_q(h + 1, 2 * (g - NG // 2) + 1)
                # PE schedule: [S.h0 + exp.h0] [PV c0,c1] [S.h1 + exp.h1]
                # [rs x2] [PV c2,c3] -- each exp's semaphore lands right
                # after its own half's matmuls, the PV/rs stream keeps the
                # PE busy while ACT runs, and nothing slow sits between the
                # PV tail and the next iteration's S matmuls.
                p_t = p_pool.tile([128, NCHUNK, QG], bf16, tag="p")
                emit_s_half(h, g, p_t, 0)
                if prev is not None:
                    emit_pv(prev_full, 0, 2)
                emit_s_half(h, g, p_t, 1)
                if g == 0:
                    # zero the fully-masked rectangles (queries < c*128 of
                    # chunk c) so the full-width rowsum reads zeros there.
                    for c in range(1, NCHUNK):
                        nc.vector.memset(p_t[:, c, 0:c * 128], 0.0)
                if prev is not None:
                    gps_q.append(emit_rs(prev_full))
                    emit_pv(prev_full, 2, NCHUNK)
                    if pg == NG - 1:
                        del head_tiles[ph]
                prev = (h, g, p_t)
            ph, pg, pp = prev
            po = o_psum.tile([128, QG], f32, tag="ops")
            pt2 = emit_t2(prev)
            prev_full = (ph, pg, pp, po, pt2)
            emit_pv(prev_full, 0, NCHUNK)
            gps_q.append(emit_rs(prev_full))
            while gps_q:
                norm_q.append(emit_back_gps(gps_q.popleft()))
            while norm_q:
                emit_back_norm(norm_q.popleft())

    nc.compile()
    return nc


def _get_compiled():
    global _COMPILED
    if _COMPILED is None:
        _COMPILED = _build()
    return _COMPILED


def _make_in_maps(query, keys, values):
    from ml_dtypes import bfloat16

    q = np.asarray(query, dtype=np.float32)
    k = np.asarray(keys, dtype=np.float32)
    v = np.asarray(values, dtype=np.float32)

    qf = q.reshape(B * H, L, D)
    kf = k.reshape(B * H, L, D)[:, L - W:, :]
    vf = v.reshape(B * H, L, D)[:, L - W:, :]

    # strictly-upper-triangular additive mask block, [q, w] indexed:
    # utri[q, w] = NEG where q < w (as matmul lhsT it lands as out[w,q] += NEG)
    utri = np.where(np.arange(128)[:, None] < np.arange(128)[None, :],
                    np.float32(NEG), np.float32(0.0)).astype(bfloat16)
    ident = np.eye(128, dtype=np.float32).astype(bfloat16)
    ones = np.ones((128, 1), dtype=np.float32).astype(bfloat16)

    in_maps = []
    for core in range(N_CORES):
        s = slice(core * HEADS_PER_CORE, (core + 1) * HEADS_PER_CORE)
        in_maps.append({
            "qT": np.ascontiguousarray(qf[s].transpose(0, 2, 1)).astype(bfloat16),
            "kT": np.ascontiguousarray(kf[s].transpose(0, 2, 1)).astype(bfloat16),
            "v": np.ascontiguousarray(vf[s]).astype(bfloat16),
            "utri": utri,
            "ident": ident,
            "ones": ones,
        })
    return in_maps


def kernel(query, keys, values, window_size):
    from concourse.bass_utils import run_bass_kernel_spmd

    w = int(window_size)
    assert np.asarray(query).shape == (B, H, L, D) and w == W

    nc = _get_compiled()
    in_maps = _make_in_maps(query, keys, values)
    res = run_bass_kernel_spmd(nc, in_maps, core_ids=list(range(N_CORES)))
    outs = [np.asarray(res.results[c]["outT"], dtype=np.float32).transpose(0, 2, 1)
            for c in range(N_CORES)]
    return np.concatenate(outs, axis=0).reshape(B, H, L, D)
